# revision 11
# baseline (speedup 1.0000x reference)
"""Trainium2 Bass kernel: unnormalized single-head attention block.

Computes, for x [4, 4096, 1024] and w_q/w_k/w_v/w_o [1024, 1024] (all fp32):
    q = x @ w_q ; k = x @ w_k ; v = x @ w_v
    scores = q @ k.T            (no softmax)
    out = (scores @ v) @ w_o

Sharding: 8 NeuronCores = (4 batches) x (2 sequence halves). Each core
computes the output rows for its 2048-row half of one batch. K/V projections
are computed for the core's own half only; the peer half is obtained with a
masked ReduceScatter over pair groups [[0,1],[2,3],[4,5],[6,7]]: each core
stages its K/V into both halves of a double buffer scaled by a host-supplied
0/1 mask (own slot zeroed), so the add-reduce-scatter delivers exactly the
peer's data into a uniform buffer on every core -- no rank-dependent
addressing in the SPMD program. Attention sums over s in (own, peer) order,
which is valid because the sum over s is order-independent.

Device math is bf16 with fp32 PSUM accumulation. The host passes the core's
half of x pre-transposed, which makes every matmul in the chain consume the
previous stage's natural layout:
    qT[e,t] = wq.T @ x.T        lhsT=wq tile,   rhs=xT
    kT[e,s] = wk.T @ x.T        lhsT=wk tile,   rhs=xT
    v[s,e]  = x @ wv            lhsT=xT tile,   rhs=wv
    sT[s,t] = k @ q.T           lhsT=kT tile,   rhs=qT
    aT[e,t] = v.T @ s           lhsT=v tile,    rhs=sT
    out[t,e]= a @ wo            lhsT=aT tile,   rhs=wo
"""

import contextlib
import ctypes
import os
import sys
import types

import numpy as np

B = 4
T = 4096
D = 1024
H = T // 2          # rows per core
P = 128             # SBUF partitions
NCORES = 8
DT = D // P         # 8 tiles along any 1024 dim
ST = T // P         # 32 tiles along the full sequence
STH = H // P        # 16 own-half s-tiles
FREE = 512          # matmul moving free dim / PSUM bank width (fp32)
SBLK = H // FREE    # 4 own-half setup s-blocks
CH = H // FREE      # 4 t-chunks per core
GROUPS = [[0, 1], [2, 3], [4, 5], [6, 7]]

_STATE = {}
LAST_RESULTS = None


def _install_axon_ntff_shim():
    """bass_utils(trace=True) under axon imports antenv.axon_hooks, which the
    agent image lacks. Provide the documented ctypes equivalent so tracing
    works; degrades to hook=None when the .so has no profile symbols."""
    try:
        import antenv.axon_hooks  # noqa: F401
        return
    except ImportError:
        pass

    so_path = "/opt/axon/libaxon_pjrt.so"

    def _make_hook():
        try:
            lib = ctypes.CDLL(so_path)
        except OSError:
            return None
        if not hasattr(lib, "axon_start_nrt_profile"):
            return None
        lib.axon_start_nrt_profile.argtypes = [
            ctypes.POINTER(ctypes.c_int64),
            ctypes.c_size_t,
        ]
        lib.axon_start_nrt_profile.restype = ctypes.c_int64
        lib.axon_stop_nrt_profile.argtypes = [ctypes.c_char_p]
        lib.axon_stop_nrt_profile.restype = ctypes.c_int64

        @contextlib.contextmanager
        def _hook(output_dir, device_ids):
            import jax

            jax.devices()
            if device_ids:
                ids = (ctypes.c_int64 * len(device_ids))(*device_ids)
                rc = lib.axon_start_nrt_profile(ids, len(device_ids))
            else:
                rc = lib.axon_start_nrt_profile(None, 0)
            if rc != 0:
                raise RuntimeError(f"axon_start_nrt_profile rc={rc}")
            try:
                yield
            finally:
                n = lib.axon_stop_nrt_profile(str(output_dir).encode())
                print(f"profile: {n} file(s) written to {output_dir}", file=sys.stderr)

        return _hook

    mod = types.ModuleType("antenv.axon_hooks")
    mod.get_axon_ntff_profile_hook = _make_hook
    mod.set_axon_ntff_profile_hook = lambda h: None
    sys.modules["antenv.axon_hooks"] = mod


def _trace_kernel(tc, xT, wq, wk, wv, wo, mask, out):
    import concourse.mybir as mybir
    from concourse.bass import ts

    nc = tc.nc
    f32 = mybir.dt.float32
    bf16 = mybir.dt.bfloat16

    with contextlib.ExitStack() as top:
        # Long-lived pools
        ktr_pool = top.enter_context(tc.tile_pool(name="ktr", bufs=DT))
        ktb_pool = top.enter_context(tc.tile_pool(name="ktb", bufs=STH))
        qt_pool = top.enter_context(tc.tile_pool(name="qt", bufs=DT))
        const_pool = top.enter_context(tc.tile_pool(name="cst", bufs=1))
        ps_pool = top.enter_context(tc.tile_pool(name="ps", bufs=4, space="PSUM"))
        ps2_pool = top.enter_context(tc.tile_pool(name="ps2", bufs=4, space="PSUM"))
        dram_pool = top.enter_context(tc.tile_pool(name="cdram", bufs=6, space="DRAM"))

        # Own-half kT in row layout (filled straight from PSUM evictions);
        # peer-half kT in st-blocked layout (one DMA per s-tile from kpeer).
        ktr = [
            ktr_pool.tile([P, H], bf16, name=f"ktr{i}", tag="ktr") for i in range(DT)
        ]
        ktb = [
            ktb_pool.tile([P, DT, P], bf16, name=f"ktb{i}", tag="ktb")
            for i in range(STH)
        ]
        qt = [qt_pool.tile([P, H], bf16, name=f"qt{i}", tag="qt") for i in range(DT)]

        mb = const_pool.tile([P, 2], f32, name="mb", tag="mb")
        nc.sync.dma_start(out=mb[:], in_=mask)

        # Collective staging (2-core groups need Local addr space).
        kstage = dram_pool.tile([2, STH, DT, P, P], bf16, name="kstage", tag="kst")
        kpeer = dram_pool.tile([STH, DT, P, P], bf16, name="kpeer", tag="kp")
        vstage = dram_pool.tile([STH, P, D], bf16, name="vstage", tag="vso")
        vstageRS = dram_pool.tile([2, STH, P, D], bf16, name="vstageRS", tag="vsr")
        vpeer = dram_pool.tile([STH, P, D], bf16, name="vpeer", tag="vp")

        # ---------------- setup: project k/v/q over own half ----------------
        with contextlib.ExitStack() as setup:
            wf_pool = setup.enter_context(tc.tile_pool(name="wf", bufs=2))
            wset_pool = setup.enter_context(tc.tile_pool(name="wset", bufs=2 * DT))
            xf_pool = setup.enter_context(tc.tile_pool(name="xf", bufs=3))
            xb_pool = setup.enter_context(tc.tile_pool(name="xb", bufs=2 * DT))
            esb_pool = setup.enter_context(tc.tile_pool(name="esb", bufs=6))

            def cast_weight(w_ap):
                tiles = []
                for i in range(DT):
                    wf = wf_pool.tile([P, D], f32, name="wf", tag="wf")
                    nc.sync.dma_start(out=wf[:], in_=w_ap[ts(i, P), :])
                    wb = wset_pool.tile([P, D], bf16, name="wb", tag="wset")
                    nc.scalar.copy(wb[:], wf[:])
                    tiles.append(wb)
                return tiles

            def load_x_block(blk):
                xb = []
                for d in range(DT):
                    xf = xf_pool.tile([P, FREE], f32, name="xf", tag="xf")
                    nc.sync.dma_start(out=xf[:], in_=xT[ts(d, P), ts(blk, FREE)])
                    xbt = xb_pool.tile([P, FREE], bf16, name="xbt", tag="xb")
                    nc.scalar.copy(xbt[:], xf[:])
                    xb.append(xbt)
                return xb

            # --- K pass (own half): fill ktr + masked staging for the RS ---
            wkb = cast_weight(wk)
            for blk in range(SBLK):
                xb = load_x_block(blk)
                for e in range(DT):
                    psum = ps_pool.tile([P, FREE], f32, name="psk", tag="ps")
                    for d in range(DT):
                        nc.tensor.matmul(
                            psum[:],
                            wkb[d][:, ts(e, P)],
                            xb[d][:],
                            start=(d == 0),
                            stop=(d == DT - 1),
                        )
                    nc.vector.tensor_copy(ktr[e][:, ts(blk, FREE)], psum[:])
                    for part in range(2):
                        km = esb_pool.tile([P, FREE], bf16, name="km", tag="esbk")
                        nc.vector.tensor_scalar_mul(
                            km[:], ktr[e][:, ts(blk, FREE)], mb[:, part : part + 1]
                        )
                        # Stores go out on the scalar HWDGE queue so they do
                        # not back up the sync queue feeding the x loads. The
                        # SBUF side of a DMA must iterate partitions first.
                        nc.scalar.dma_start(
                            out=kstage[part, ts(blk, 4), e].rearrange(
                                "a p f -> p a f"
                            ),
                            in_=km.rearrange("p (a f) -> p a f", a=4),
                        )
            nc.gpsimd.collective_compute(
                "ReduceScatter",
                mybir.AluOpType.add,
                replica_groups=GROUPS,
                ins=[kstage.opt()],
                outs=[kpeer.opt()],
            )

            # --- combined V+Q pass (own half): one x load per block ---
            wvb = cast_weight(wv)
            wqb = cast_weight(wq)
            for blk in range(SBLK):
                xb = load_x_block(blk)
                for ss in range(FREE // P):
                    s_local = blk * (FREE // P) + ss
                    vt = esb_pool.tile([P, D], bf16, name="vt", tag="esbv")
                    for nh in range(2):
                        psum = ps2_pool.tile([P, FREE], f32, name="psv", tag="ps2")
                        for d in range(DT):
                            nc.tensor.matmul(
                                psum[:],
                                xb[d][:, ts(ss, P)],
                                wvb[d][:, ts(nh, FREE)],
                                start=(d == 0),
                                stop=(d == DT - 1),
                            )
                        nc.vector.tensor_copy(vt[:, ts(nh, FREE)], psum[:])
                    nc.scalar.dma_start(out=vstage[s_local], in_=vt[:])
                    for part in range(2):
                        vm = esb_pool.tile([P, D], bf16, name="vm", tag="esbm")
                        nc.vector.tensor_scalar_mul(
                            vm[:], vt[:], mb[:, part : part + 1]
                        )
                        nc.scalar.dma_start(out=vstageRS[part, s_local], in_=vm[:])
                for e in range(DT):
                    psum = ps_pool.tile([P, FREE], f32, name="psq", tag="ps")
                    for d in range(DT):
                        nc.tensor.matmul(
                            psum[:],
                            wqb[d][:, ts(e, P)],
                            xb[d][:],
                            start=(d == 0),
                            stop=(d == DT - 1),
                        )
                    nc.vector.tensor_copy(qt[e][:, ts(blk, FREE)], psum[:])
            nc.gpsimd.collective_compute(
                "ReduceScatter",
                mybir.AluOpType.add,
                replica_groups=GROUPS,
                ins=[vstageRS.opt()],
                outs=[vpeer.opt()],
            )

        # Peer-half kT into SBUF (waits on the K ReduceScatter via tile deps)
        for st in range(STH):
            nc.sync.dma_start(
                out=ktb[st][:], in_=kpeer[st].rearrange("e p f -> p e f")
            )

        # w_o cast (after setup pools release)
        wo_pool = top.enter_context(tc.tile_pool(name="wob", bufs=DT))
        wof_pool = top.enter_context(tc.tile_pool(name="wof", bufs=2))
        wob = []
        for i in range(DT):
            wf = wof_pool.tile([P, D], f32, name="wof", tag="wof")
            nc.sync.dma_start(out=wf[:], in_=wo[ts(i, P), :])
            wb = wo_pool.tile([P, D], bf16, name="wob", tag="wob")
            nc.scalar.copy(wb[:], wf[:])
            wob.append(wb)

        # ---------------- main loop over t-chunks ----------------
        sct_pool = top.enter_context(tc.tile_pool(name="sct", bufs=ST))
        att_pool = top.enter_context(tc.tile_pool(name="att", bufs=2 * DT))
        vld_pool = top.enter_context(tc.tile_pool(name="vld", bufs=6))
        ost_pool = top.enter_context(tc.tile_pool(name="ost", bufs=4))

        for c in range(CH):
            # scores^T [s, t-chunk]: own half from ktr, peer half from ktb
            sct = []
            for st in range(ST):
                psum = ps_pool.tile([P, FREE], f32, name="pss", tag="ps")
                for e in range(DT):
                    lhsT = (
                        ktr[e][:, ts(st, P)]
                        if st < STH
                        else ktb[st - STH][:, e, :]
                    )
                    nc.tensor.matmul(
                        psum[:],
                        lhsT,
                        qt[e][:, ts(c, FREE)],
                        start=(e == 0),
                        stop=(e == DT - 1),
                    )
                sc = sct_pool.tile([P, FREE], bf16, name="sc", tag="sct")
                nc.vector.tensor_copy(sc[:], psum[:])
                sct.append(sc)

            # attn^T [e, t-chunk], accumulated over s in 2 e-groups of 4;
            # own-half v from vstage (local), peer half from vpeer
            att = [None] * DT
            for g in range(2):
                accs = [
                    ps2_pool.tile([P, FREE], f32, name=f"acc{j}", tag="ps2")
                    for j in range(4)
                ]
                for st in range(ST):
                    vt = vld_pool.tile([P, D], bf16, name="vl", tag="vld")
                    src = vstage[st] if st < STH else vpeer[st - STH]
                    nc.sync.dma_start(out=vt[:], in_=src)
                    for j in range(4):
                        e = g * 4 + j
                        nc.tensor.matmul(
                            accs[j][:],
                            vt[:, ts(e, P)],
                            sct[st][:],
                            start=(st == 0),
                            stop=(st == ST - 1),
                        )
                for j in range(4):
                    a = att_pool.tile([P, FREE], bf16, name="at", tag="att")
                    nc.vector.tensor_copy(a[:], accs[j][:])
                    att[g * 4 + j] = a

            # output projection [t-chunk, 1024]
            for tt in range(FREE // P):
                for nh in range(2):
                    psum = ps_pool.tile([P, FREE], f32, name="pso", tag="ps")
                    for e in range(DT):
                        nc.tensor.matmul(
                            psum[:],
                            att[e][:, ts(tt, P)],
                            wob[e][:, ts(nh, FREE)],
                            start=(e == 0),
                            stop=(e == DT - 1),
                        )
                    ot = ost_pool.tile([P, FREE], f32, name="ot", tag="ost")
                    nc.scalar.copy(ot[:], psum[:])
                    row = c * FREE + tt * P
                    nc.scalar.dma_start(
                        out=out[row : row + P, ts(nh, FREE)], in_=ot[:]
                    )


def _load_const(tc, dram_ap, pool):
    import concourse.mybir as mybir

    nc = tc.nc
    t = pool.tile(list(dram_ap.shape), mybir.dt.float32, name="mf", tag="mf")
    nc.sync.dma_start(out=t[:], in_=dram_ap)
    return t[:]


def _build():
    _install_axon_ntff_shim()
    import concourse.mybir as mybir
    import concourse.tile as tile
    from concourse import bacc

    f32 = mybir.dt.float32
    nc = bacc.Bacc("TRN2", target_bir_lowering=False, debug=False, num_devices=NCORES)
    xT = nc.dram_tensor("xT", [D, H], f32, kind="ExternalInput").ap()
    wq = nc.dram_tensor("wq", [D, D], f32, kind="ExternalInput").ap()
    wk = nc.dram_tensor("wk", [D, D], f32, kind="ExternalInput").ap()
    wv = nc.dram_tensor("wv", [D, D], f32, kind="ExternalInput").ap()
    wo = nc.dram_tensor("wo", [D, D], f32, kind="ExternalInput").ap()
    mask = nc.dram_tensor("mask", [P, 2], f32, kind="ExternalInput").ap()
    out = nc.dram_tensor("out", [H, D], f32, kind="ExternalOutput").ap()

    with tile.TileContext(nc) as tc:
        _trace_kernel(tc, xT, wq, wk, wv, wo, mask, out)
    nc.compile()
    return nc


def kernel(x, w_q, w_k, w_v, w_o):
    global LAST_RESULTS
    from concourse import bass_utils

    if "nc" not in _STATE:
        _STATE["nc"] = _build()
    nc = _STATE["nc"]

    x = np.ascontiguousarray(x, dtype=np.float32)
    in_maps = []
    for core in range(NCORES):
        b, half = core // 2, core % 2
        xT = np.ascontiguousarray(x[b, half * H : (half + 1) * H].T)
        mask = np.zeros((P, 2), dtype=np.float32)
        mask[:, 1 - half] = 1.0  # zero own slot; pair position == half
        in_maps.append(
            {
                "xT": xT,
                "wq": np.ascontiguousarray(w_q, dtype=np.float32),
                "wk": np.ascontiguousarray(w_k, dtype=np.float32),
                "wv": np.ascontiguousarray(w_v, dtype=np.float32),
                "wo": np.ascontiguousarray(w_o, dtype=np.float32),
                "mask": mask,
            }
        )

    LAST_RESULTS = bass_utils.run_bass_kernel_spmd(
        nc, in_maps, core_ids=list(range(NCORES))
    )
    out = np.empty((B, T, D), dtype=np.float32)
    for core in range(NCORES):
        b, half = core // 2, core % 2
        out[b, half * H : (half + 1) * H] = LAST_RESULTS.results[core]["out"]
    return out


# revision 12
# speedup vs baseline: 1.0962x; 1.0962x over previous
"""Trainium2 Bass kernel: unnormalized single-head attention block.

Computes, for x [4, 4096, 1024] and w_q/w_k/w_v/w_o [1024, 1024] (all fp32):
    q = x @ w_q ; k = x @ w_k ; v = x @ w_v
    scores = q @ k.T            (no softmax)
    out = (scores @ v) @ w_o

Sharding: 8 NeuronCores = (4 batches) x (2 sequence halves). Each core
computes the output rows for its 2048-row half of one batch. The host passes
x.T with the core's own half first ("rotated" column order); attention sums
over s are order-independent, so all per-core tensors use that rotated
order consistently.

K projection is computed for the own half only; the peer half arrives via a
masked ReduceScatter over pair groups [[0,1],[2,3],[4,5],[6,7]]: each core
stages its K into both halves of a double buffer scaled by a host-supplied
0/1 mask (own slot zeroed), so the add-reduce-scatter delivers exactly the
peer's K into a uniform buffer on every core -- no rank-dependent addressing
in the SPMD program. V is cheaper to recompute than to exchange on this
fabric (collective transfers are ~100us for 4MB and serialize on the CC
core), so each core projects V over the full rotated sequence.

Device math is bf16 with fp32 PSUM accumulation. Layout chaining (no
on-device transposes anywhere):
    qT[e,t] = wq.T @ x.T        lhsT=wq tile,   rhs=xT
    kT[e,s] = wk.T @ x.T        lhsT=wk tile,   rhs=xT
    v[s,e]  = x @ wv            lhsT=xT tile,   rhs=wv
    sT[s,t] = k @ q.T           lhsT=kT tile,   rhs=qT
    aT[e,t] = v.T @ s           lhsT=v tile,    rhs=sT
    out[t,e]= a @ wo            lhsT=aT tile,   rhs=wo
"""

import contextlib
import ctypes
import os
import sys
import types

import numpy as np

B = 4
T = 4096
D = 1024
H = T // 2          # rows per core
P = 128             # SBUF partitions
NCORES = 8
DT = D // P         # 8 tiles along any 1024 dim
ST = T // P         # 32 tiles along the full sequence
STH = H // P        # 16 own-half s-tiles
FREE = 512          # matmul moving free dim / PSUM bank width (fp32)
SBLK = T // FREE    # 8 full-sequence blocks of 512
CH = H // FREE      # 4 t-chunks per core
GROUPS = [[0, 1], [2, 3], [4, 5], [6, 7]]

_STATE = {}
LAST_RESULTS = None


def _install_axon_ntff_shim():
    """bass_utils(trace=True) under axon imports antenv.axon_hooks, which the
    agent image lacks. Provide the documented ctypes equivalent so tracing
    works; degrades to hook=None when the .so has no profile symbols."""
    try:
        import antenv.axon_hooks  # noqa: F401
        return
    except ImportError:
        pass

    so_path = "/opt/axon/libaxon_pjrt.so"

    def _make_hook():
        try:
            lib = ctypes.CDLL(so_path)
        except OSError:
            return None
        if not hasattr(lib, "axon_start_nrt_profile"):
            return None
        lib.axon_start_nrt_profile.argtypes = [
            ctypes.POINTER(ctypes.c_int64),
            ctypes.c_size_t,
        ]
        lib.axon_start_nrt_profile.restype = ctypes.c_int64
        lib.axon_stop_nrt_profile.argtypes = [ctypes.c_char_p]
        lib.axon_stop_nrt_profile.restype = ctypes.c_int64

        @contextlib.contextmanager
        def _hook(output_dir, device_ids):
            import jax

            jax.devices()
            if device_ids:
                ids = (ctypes.c_int64 * len(device_ids))(*device_ids)
                rc = lib.axon_start_nrt_profile(ids, len(device_ids))
            else:
                rc = lib.axon_start_nrt_profile(None, 0)
            if rc != 0:
                raise RuntimeError(f"axon_start_nrt_profile rc={rc}")
            try:
                yield
            finally:
                n = lib.axon_stop_nrt_profile(str(output_dir).encode())
                print(f"profile: {n} file(s) written to {output_dir}", file=sys.stderr)

        return _hook

    mod = types.ModuleType("antenv.axon_hooks")
    mod.get_axon_ntff_profile_hook = _make_hook
    mod.set_axon_ntff_profile_hook = lambda h: None
    sys.modules["antenv.axon_hooks"] = mod


def _trace_kernel(tc, xT, wq, wk, wv, wo, mask, out):
    import concourse.mybir as mybir
    from concourse.bass import ts

    nc = tc.nc
    f32 = mybir.dt.float32
    bf16 = mybir.dt.bfloat16

    with contextlib.ExitStack() as top:
        # Long-lived pools
        ktr_pool = top.enter_context(tc.tile_pool(name="ktr", bufs=DT))
        ktb_pool = top.enter_context(tc.tile_pool(name="ktb", bufs=STH))
        qt_pool = top.enter_context(tc.tile_pool(name="qt", bufs=DT))
        const_pool = top.enter_context(tc.tile_pool(name="cst", bufs=1))
        ps_pool = top.enter_context(tc.tile_pool(name="ps", bufs=4, space="PSUM"))
        ps2_pool = top.enter_context(tc.tile_pool(name="ps2", bufs=4, space="PSUM"))
        dram_pool = top.enter_context(tc.tile_pool(name="cdram", bufs=4, space="DRAM"))

        # Own-half kT in row layout (filled straight from PSUM evictions);
        # peer-half kT in st-blocked layout (one DMA per s-tile from kpeer).
        ktr = [
            ktr_pool.tile([P, H], bf16, name=f"ktr{i}", tag="ktr") for i in range(DT)
        ]
        ktb = [
            ktb_pool.tile([P, DT, P], bf16, name=f"ktb{i}", tag="ktb")
            for i in range(STH)
        ]
        qt = [qt_pool.tile([P, H], bf16, name=f"qt{i}", tag="qt") for i in range(DT)]

        mb = const_pool.tile([P, 2], f32, name="mb", tag="mb")
        nc.sync.dma_start(out=mb[:], in_=mask)

        # K-collective staging (2-core groups need Local addr space) and the
        # full-sequence V staging in local DRAM.
        kstage = dram_pool.tile([2, STH, DT, P, P], bf16, name="kstage", tag="kst")
        kpeer = dram_pool.tile([STH, DT, P, P], bf16, name="kpeer", tag="kp")
        vstage = dram_pool.tile([ST, P, D], bf16, name="vstage", tag="vso")

        # ---------------- setup ----------------
        with contextlib.ExitStack() as setup:
            wf_pool = setup.enter_context(tc.tile_pool(name="wf", bufs=2))
            wset_pool = setup.enter_context(tc.tile_pool(name="wset", bufs=2 * DT))
            xf_pool = setup.enter_context(tc.tile_pool(name="xf", bufs=3))
            xb_pool = setup.enter_context(tc.tile_pool(name="xb", bufs=2 * DT))
            esb_pool = setup.enter_context(tc.tile_pool(name="esb", bufs=6))

            def cast_weight(w_ap):
                tiles = []
                for i in range(DT):
                    wf = wf_pool.tile([P, D], f32, name="wf", tag="wf")
                    nc.sync.dma_start(out=wf[:], in_=w_ap[ts(i, P), :])
                    wb = wset_pool.tile([P, D], bf16, name="wb", tag="wset")
                    nc.scalar.copy(wb[:], wf[:])
                    tiles.append(wb)
                return tiles

            def load_x_block(blk):
                xb = []
                for d in range(DT):
                    xf = xf_pool.tile([P, FREE], f32, name="xf", tag="xf")
                    nc.sync.dma_start(out=xf[:], in_=xT[ts(d, P), ts(blk, FREE)])
                    xbt = xb_pool.tile([P, FREE], bf16, name="xbt", tag="xb")
                    nc.scalar.copy(xbt[:], xf[:])
                    xb.append(xbt)
                return xb

            # --- K pass (own half): fill ktr + masked staging for the RS ---
            wkb = cast_weight(wk)
            for blk in range(CH):
                xb = load_x_block(blk)
                for e in range(DT):
                    psum = ps_pool.tile([P, FREE], f32, name="psk", tag="ps")
                    for d in range(DT):
                        nc.tensor.matmul(
                            psum[:],
                            wkb[d][:, ts(e, P)],
                            xb[d][:],
                            start=(d == 0),
                            stop=(d == DT - 1),
                        )
                    nc.vector.tensor_copy(ktr[e][:, ts(blk, FREE)], psum[:])
                    for part in range(2):
                        km = esb_pool.tile([P, FREE], bf16, name="km", tag="esbk")
                        nc.vector.tensor_scalar_mul(
                            km[:], ktr[e][:, ts(blk, FREE)], mb[:, part : part + 1]
                        )
                        # Stores ride the scalar HWDGE queue so they do not
                        # back up the sync queue feeding the x loads. The
                        # SBUF side of a DMA iterates partitions first.
                        nc.scalar.dma_start(
                            out=kstage[part, ts(blk, 4), e].rearrange(
                                "a p f -> p a f"
                            ),
                            in_=km.rearrange("p (a f) -> p a f", a=4),
                        )
            nc.gpsimd.collective_compute(
                "ReduceScatter",
                mybir.AluOpType.add,
                replica_groups=GROUPS,
                ins=[kstage.opt()],
                outs=[kpeer.opt()],
            )

            # --- combined V (full sequence) + Q (own half) pass ---
            wvb = cast_weight(wv)
            wqb = cast_weight(wq)
            for blk in range(SBLK):
                xb = load_x_block(blk)
                for ss in range(FREE // P):
                    s_tile = blk * (FREE // P) + ss
                    vt = esb_pool.tile([P, D], bf16, name="vt", tag="esbv")
                    for nh in range(2):
                        psum = ps2_pool.tile([P, FREE], f32, name="psv", tag="ps2")
                        for d in range(DT):
                            nc.tensor.matmul(
                                psum[:],
                                xb[d][:, ts(ss, P)],
                                wvb[d][:, ts(nh, FREE)],
                                start=(d == 0),
                                stop=(d == DT - 1),
                            )
                        nc.vector.tensor_copy(vt[:, ts(nh, FREE)], psum[:])
                    nc.scalar.dma_start(out=vstage[s_tile], in_=vt[:])
                if blk < CH:  # q projection for the own half
                    for e in range(DT):
                        psum = ps_pool.tile([P, FREE], f32, name="psq", tag="ps")
                        for d in range(DT):
                            nc.tensor.matmul(
                                psum[:],
                                wqb[d][:, ts(e, P)],
                                xb[d][:],
                                start=(d == 0),
                                stop=(d == DT - 1),
                            )
                        nc.vector.tensor_copy(qt[e][:, ts(blk, FREE)], psum[:])

        # Peer-half kT into SBUF (waits on the K ReduceScatter via tile deps)
        for st in range(STH):
            nc.sync.dma_start(
                out=ktb[st][:], in_=kpeer[st].rearrange("e p f -> p e f")
            )

        # w_o cast (after setup pools release)
        wo_pool = top.enter_context(tc.tile_pool(name="wob", bufs=DT))
        wof_pool = top.enter_context(tc.tile_pool(name="wof", bufs=2))
        wob = []
        for i in range(DT):
            wf = wof_pool.tile([P, D], f32, name="wof", tag="wof")
            nc.sync.dma_start(out=wf[:], in_=wo[ts(i, P), :])
            wb = wo_pool.tile([P, D], bf16, name="wob", tag="wob")
            nc.scalar.copy(wb[:], wf[:])
            wob.append(wb)

        # ---------------- main loop over t-chunks ----------------
        sct_pool = top.enter_context(tc.tile_pool(name="sct", bufs=ST))
        att_pool = top.enter_context(tc.tile_pool(name="att", bufs=2 * DT))
        vld_pool = top.enter_context(tc.tile_pool(name="vld", bufs=6))
        ost_pool = top.enter_context(tc.tile_pool(name="ost", bufs=4))

        for c in range(CH):
            # scores^T [s, t-chunk]: own half from ktr, peer half from ktb
            sct = []
            for st in range(ST):
                psum = ps_pool.tile([P, FREE], f32, name="pss", tag="ps")
                for e in range(DT):
                    lhsT = (
                        ktr[e][:, ts(st, P)]
                        if st < STH
                        else ktb[st - STH][:, e, :]
                    )
                    nc.tensor.matmul(
                        psum[:],
                        lhsT,
                        qt[e][:, ts(c, FREE)],
                        start=(e == 0),
                        stop=(e == DT - 1),
                    )
                sc = sct_pool.tile([P, FREE], bf16, name="sc", tag="sct")
                nc.vector.tensor_copy(sc[:], psum[:])
                sct.append(sc)

            # attn^T [e, t-chunk], accumulated over s in 2 e-groups of 4
            att = [None] * DT
            for g in range(2):
                accs = [
                    ps2_pool.tile([P, FREE], f32, name=f"acc{j}", tag="ps2")
                    for j in range(4)
                ]
                for st in range(ST):
                    vt = vld_pool.tile([P, D], bf16, name="vl", tag="vld")
                    nc.sync.dma_start(out=vt[:], in_=vstage[st])
                    for j in range(4):
                        e = g * 4 + j
                        nc.tensor.matmul(
                            accs[j][:],
                            vt[:, ts(e, P)],
                            sct[st][:],
                            start=(st == 0),
                            stop=(st == ST - 1),
                        )
                for j in range(4):
                    a = att_pool.tile([P, FREE], bf16, name="at", tag="att")
                    nc.vector.tensor_copy(a[:], accs[j][:])
                    att[g * 4 + j] = a

            # output projection [t-chunk, 1024]
            for tt in range(FREE // P):
                for nh in range(2):
                    psum = ps_pool.tile([P, FREE], f32, name="pso", tag="ps")
                    for e in range(DT):
                        nc.tensor.matmul(
                            psum[:],
                            att[e][:, ts(tt, P)],
                            wob[e][:, ts(nh, FREE)],
                            start=(e == 0),
                            stop=(e == DT - 1),
                        )
                    ot = ost_pool.tile([P, FREE], f32, name="ot", tag="ost")
                    nc.scalar.copy(ot[:], psum[:])
                    row = c * FREE + tt * P
                    nc.scalar.dma_start(
                        out=out[row : row + P, ts(nh, FREE)], in_=ot[:]
                    )


def _build():
    _install_axon_ntff_shim()
    import concourse.mybir as mybir
    import concourse.tile as tile
    from concourse import bacc

    f32 = mybir.dt.float32
    nc = bacc.Bacc("TRN2", target_bir_lowering=False, debug=False, num_devices=NCORES)
    xT = nc.dram_tensor("xT", [D, T], f32, kind="ExternalInput").ap()
    wq = nc.dram_tensor("wq", [D, D], f32, kind="ExternalInput").ap()
    wk = nc.dram_tensor("wk", [D, D], f32, kind="ExternalInput").ap()
    wv = nc.dram_tensor("wv", [D, D], f32, kind="ExternalInput").ap()
    wo = nc.dram_tensor("wo", [D, D], f32, kind="ExternalInput").ap()
    mask = nc.dram_tensor("mask", [P, 2], f32, kind="ExternalInput").ap()
    out = nc.dram_tensor("out", [H, D], f32, kind="ExternalOutput").ap()

    with tile.TileContext(nc) as tc:
        _trace_kernel(tc, xT, wq, wk, wv, wo, mask, out)
    nc.compile()
    return nc


def kernel(x, w_q, w_k, w_v, w_o):
    global LAST_RESULTS
    from concourse import bass_utils

    if "nc" not in _STATE:
        _STATE["nc"] = _build()
    nc = _STATE["nc"]

    x = np.ascontiguousarray(x, dtype=np.float32)
    in_maps = []
    for core in range(NCORES):
        b, half = core // 2, core % 2
        own = x[b, half * H : (half + 1) * H]
        oth = x[b, (1 - half) * H : (2 - half) * H]
        xT = np.ascontiguousarray(np.concatenate([own, oth], axis=0).T)
        m = np.zeros((P, 2), dtype=np.float32)
        m[:, 1 - half] = 1.0  # zero own slot; pair position == half
        in_maps.append(
            {
                "xT": xT,
                "wq": np.ascontiguousarray(w_q, dtype=np.float32),
                "wk": np.ascontiguousarray(w_k, dtype=np.float32),
                "wv": np.ascontiguousarray(w_v, dtype=np.float32),
                "wo": np.ascontiguousarray(w_o, dtype=np.float32),
                "mask": m,
            }
        )

    LAST_RESULTS = bass_utils.run_bass_kernel_spmd(
        nc, in_maps, core_ids=list(range(NCORES))
    )
    out = np.empty((B, T, D), dtype=np.float32)
    for core in range(NCORES):
        b, half = core // 2, core % 2
        out[b, half * H : (half + 1) * H] = LAST_RESULTS.results[core]["out"]
    return out


# revision 14
# speedup vs baseline: 1.1520x; 1.0510x over previous
"""Trainium2 Bass kernel: unnormalized single-head attention block.

Computes, for x [4, 4096, 1024] and w_q/w_k/w_v/w_o [1024, 1024] (all fp32):
    q = x @ w_q ; k = x @ w_k ; v = x @ w_v
    scores = q @ k.T            (no softmax)
    out = (scores @ v) @ w_o

Sharding: 8 NeuronCores = (4 batches) x (2 sequence halves). Each core
computes the output rows for its 2048-row half of one batch. The host passes
x.T with the core's own half first ("rotated" column order); attention sums
over s are order-independent, so all per-core tensors use that rotated
order consistently.

K projection is computed for the own half only; the peer half arrives via a
masked ReduceScatter over pair groups [[0,1],[2,3],[4,5],[6,7]]: each core
stages its K into both halves of a double buffer scaled by a host-supplied
0/1 mask (own slot zeroed), so the add-reduce-scatter delivers exactly the
peer's K into a uniform buffer on every core -- no rank-dependent addressing
in the SPMD program. V is cheaper to recompute than to exchange on this
fabric (collective transfers are ~100us for 4MB and serialize on the CC
core), so each core projects V over the full rotated sequence.

Device math is bf16 with fp32 PSUM accumulation. Layout chaining (no
on-device transposes anywhere):
    qT[e,t] = wq.T @ x.T        lhsT=wq tile,   rhs=xT
    kT[e,s] = wk.T @ x.T        lhsT=wk tile,   rhs=xT
    v[s,e]  = x @ wv            lhsT=xT tile,   rhs=wv
    sT[s,t] = k @ q.T           lhsT=kT tile,   rhs=qT
    aT[e,t] = v.T @ s           lhsT=v tile,    rhs=sT
    out[t,e]= a @ wo            lhsT=aT tile,   rhs=wo
"""

import contextlib
import ctypes
import os
import sys
import types

import numpy as np

B = 4
T = 4096
D = 1024
H = T // 2          # rows per core
P = 128             # SBUF partitions
NCORES = 8
DT = D // P         # 8 tiles along any 1024 dim
ST = T // P         # 32 tiles along the full sequence
STH = H // P        # 16 own-half s-tiles
FREE = 512          # matmul moving free dim / PSUM bank width (fp32)
SBLK = T // FREE    # 8 full-sequence blocks of 512
CH = H // FREE      # 4 t-chunks per core
GROUPS = [[0, 1], [2, 3], [4, 5], [6, 7]]

_STATE = {}
LAST_RESULTS = None


def _install_axon_ntff_shim():
    """bass_utils(trace=True) under axon imports antenv.axon_hooks, which the
    agent image lacks. Provide the documented ctypes equivalent so tracing
    works; degrades to hook=None when the .so has no profile symbols."""
    try:
        import antenv.axon_hooks  # noqa: F401
        return
    except ImportError:
        pass

    so_path = "/opt/axon/libaxon_pjrt.so"

    def _make_hook():
        try:
            lib = ctypes.CDLL(so_path)
        except OSError:
            return None
        if not hasattr(lib, "axon_start_nrt_profile"):
            return None
        lib.axon_start_nrt_profile.argtypes = [
            ctypes.POINTER(ctypes.c_int64),
            ctypes.c_size_t,
        ]
        lib.axon_start_nrt_profile.restype = ctypes.c_int64
        lib.axon_stop_nrt_profile.argtypes = [ctypes.c_char_p]
        lib.axon_stop_nrt_profile.restype = ctypes.c_int64

        @contextlib.contextmanager
        def _hook(output_dir, device_ids):
            import jax

            jax.devices()
            if device_ids:
                ids = (ctypes.c_int64 * len(device_ids))(*device_ids)
                rc = lib.axon_start_nrt_profile(ids, len(device_ids))
            else:
                rc = lib.axon_start_nrt_profile(None, 0)
            if rc != 0:
                raise RuntimeError(f"axon_start_nrt_profile rc={rc}")
            try:
                yield
            finally:
                n = lib.axon_stop_nrt_profile(str(output_dir).encode())
                print(f"profile: {n} file(s) written to {output_dir}", file=sys.stderr)

        return _hook

    mod = types.ModuleType("antenv.axon_hooks")
    mod.get_axon_ntff_profile_hook = _make_hook
    mod.set_axon_ntff_profile_hook = lambda h: None
    sys.modules["antenv.axon_hooks"] = mod


def _trace_kernel(tc, xT, wq, wk, wv, wo, mask, out):
    import concourse.mybir as mybir
    from concourse.bass import ts

    nc = tc.nc
    f32 = mybir.dt.float32
    bf16 = mybir.dt.bfloat16

    with contextlib.ExitStack() as top:
        # Long-lived pools
        ktr_pool = top.enter_context(tc.tile_pool(name="ktr", bufs=DT))
        ktb_pool = top.enter_context(tc.tile_pool(name="ktb", bufs=DT))
        qt_pool = top.enter_context(tc.tile_pool(name="qt", bufs=DT))
        const_pool = top.enter_context(tc.tile_pool(name="cst", bufs=1))
        ps_pool = top.enter_context(tc.tile_pool(name="ps", bufs=4, space="PSUM"))
        ps2_pool = top.enter_context(tc.tile_pool(name="ps2", bufs=4, space="PSUM"))
        dram_pool = top.enter_context(tc.tile_pool(name="cdram", bufs=4, space="DRAM"))

        # Own-half kT in row layout (filled straight from PSUM evictions);
        # peer-half kT in st-blocked layout (one DMA per s-tile from kpeer).
        ktr = [
            ktr_pool.tile([P, H], bf16, name=f"ktr{i}", tag="ktr") for i in range(DT)
        ]
        ktb = [
            ktb_pool.tile([P, H], bf16, name=f"ktb{i}", tag="ktb") for i in range(DT)
        ]
        qt = [qt_pool.tile([P, H], bf16, name=f"qt{i}", tag="qt") for i in range(DT)]

        mb = const_pool.tile([P, 2], f32, name="mb", tag="mb")
        nc.sync.dma_start(out=mb[:], in_=mask)

        # K-collective staging (2-core groups need Local addr space) and the
        # full-sequence V staging in local DRAM.
        kstage = dram_pool.tile([2, DT, P, H], bf16, name="kstage", tag="kst")
        kpeer = dram_pool.tile([DT, P, H], bf16, name="kpeer", tag="kp")
        vstage = dram_pool.tile([ST, P, D], bf16, name="vstage", tag="vso")

        # ---------------- setup ----------------
        with contextlib.ExitStack() as setup:
            wf_pool = setup.enter_context(tc.tile_pool(name="wf", bufs=2))
            wset_pool = setup.enter_context(tc.tile_pool(name="wset", bufs=2 * DT))
            xf_pool = setup.enter_context(tc.tile_pool(name="xf", bufs=3))
            xb_pool = setup.enter_context(tc.tile_pool(name="xb", bufs=2 * DT))
            esb_pool = setup.enter_context(tc.tile_pool(name="esb", bufs=6))

            def cast_weight(w_ap):
                tiles = []
                for i in range(DT):
                    wf = wf_pool.tile([P, D], f32, name="wf", tag="wf")
                    nc.sync.dma_start(out=wf[:], in_=w_ap[ts(i, P), :])
                    wb = wset_pool.tile([P, D], bf16, name="wb", tag="wset")
                    nc.scalar.copy(wb[:], wf[:])
                    tiles.append(wb)
                return tiles

            def load_x_block(blk):
                xb = []
                for d in range(DT):
                    xf = xf_pool.tile([P, FREE], f32, name="xf", tag="xf")
                    nc.sync.dma_start(out=xf[:], in_=xT[ts(d, P), ts(blk, FREE)])
                    xbt = xb_pool.tile([P, FREE], bf16, name="xbt", tag="xb")
                    nc.scalar.copy(xbt[:], xf[:])
                    xb.append(xbt)
                return xb

            # --- K pass (own half): fill ktr + masked staging for the RS ---
            wkb = cast_weight(wk)
            for blk in range(CH):
                xb = load_x_block(blk)
                for e in range(DT):
                    psum = ps_pool.tile([P, FREE], f32, name="psk", tag="ps")
                    for d in range(DT):
                        nc.tensor.matmul(
                            psum[:],
                            wkb[d][:, ts(e, P)],
                            xb[d][:],
                            start=(d == 0),
                            stop=(d == DT - 1),
                        )
                    nc.vector.tensor_copy(ktr[e][:, ts(blk, FREE)], psum[:])
                    for part in range(2):
                        km = esb_pool.tile([P, FREE], bf16, name="km", tag="esbk")
                        nc.vector.tensor_scalar_mul(
                            km[:], ktr[e][:, ts(blk, FREE)], mb[:, part : part + 1]
                        )
                        # Stores ride the scalar HWDGE queue so they do not
                        # back up the sync queue feeding the x loads.
                        nc.scalar.dma_start(
                            out=kstage[part, e, :, ts(blk, FREE)], in_=km[:]
                        )
            nc.gpsimd.collective_compute(
                "ReduceScatter",
                mybir.AluOpType.add,
                replica_groups=GROUPS,
                ins=[kstage.opt()],
                outs=[kpeer.opt()],
            )

            # --- combined V (full sequence) + Q (own half) pass ---
            wvb = cast_weight(wv)
            wqb = cast_weight(wq)
            for blk in range(SBLK):
                xb = load_x_block(blk)
                for ss in range(FREE // P):
                    s_tile = blk * (FREE // P) + ss
                    vt = esb_pool.tile([P, D], bf16, name="vt", tag="esbv")
                    for nh in range(2):
                        psum = ps2_pool.tile([P, FREE], f32, name="psv", tag="ps2")
                        for d in range(DT):
                            nc.tensor.matmul(
                                psum[:],
                                xb[d][:, ts(ss, P)],
                                wvb[d][:, ts(nh, FREE)],
                                start=(d == 0),
                                stop=(d == DT - 1),
                            )
                        nc.vector.tensor_copy(vt[:, ts(nh, FREE)], psum[:])
                    nc.scalar.dma_start(out=vstage[s_tile], in_=vt[:])
                if blk < CH:  # q projection for the own half
                    for e in range(DT):
                        psum = ps_pool.tile([P, FREE], f32, name="psq", tag="ps")
                        for d in range(DT):
                            nc.tensor.matmul(
                                psum[:],
                                wqb[d][:, ts(e, P)],
                                xb[d][:],
                                start=(d == 0),
                                stop=(d == DT - 1),
                            )
                        nc.vector.tensor_copy(qt[e][:, ts(blk, FREE)], psum[:])

        # Peer-half kT into SBUF (waits on the K ReduceScatter via tile
        # deps). Issued from the otherwise-idle SWDGE queue so the wait on
        # the collective cannot stall the sync/scalar DMA queues.
        for e in range(DT):
            nc.gpsimd.dma_start(out=ktb[e][:], in_=kpeer[e])

        # w_o cast (after setup pools release)
        wo_pool = top.enter_context(tc.tile_pool(name="wob", bufs=DT))
        wof_pool = top.enter_context(tc.tile_pool(name="wof", bufs=2))
        wob = []
        for i in range(DT):
            wf = wof_pool.tile([P, D], f32, name="wof", tag="wof")
            nc.sync.dma_start(out=wf[:], in_=wo[ts(i, P), :])
            wb = wo_pool.tile([P, D], bf16, name="wob", tag="wob")
            nc.scalar.copy(wb[:], wf[:])
            wob.append(wb)

        # ---------------- main loop over t-chunks ----------------
        sct_pool = top.enter_context(tc.tile_pool(name="sct", bufs=ST))
        att_pool = top.enter_context(tc.tile_pool(name="att", bufs=2 * DT))
        vld_pool = top.enter_context(tc.tile_pool(name="vld", bufs=6))
        ost_pool = top.enter_context(tc.tile_pool(name="ost", bufs=4))

        for c in range(CH):
            # scores^T [s, t-chunk]: own half from ktr, peer half from ktb
            sct = []
            for st in range(ST):
                psum = ps_pool.tile([P, FREE], f32, name="pss", tag="ps")
                for e in range(DT):
                    lhsT = (
                        ktr[e][:, ts(st, P)]
                        if st < STH
                        else ktb[e][:, ts(st - STH, P)]
                    )
                    nc.tensor.matmul(
                        psum[:],
                        lhsT,
                        qt[e][:, ts(c, FREE)],
                        start=(e == 0),
                        stop=(e == DT - 1),
                    )
                sc = sct_pool.tile([P, FREE], bf16, name="sc", tag="sct")
                nc.vector.tensor_copy(sc[:], psum[:])
                sct.append(sc)

            # attn^T [e, t-chunk], accumulated over s in 2 e-groups of 4
            att = [None] * DT
            for g in range(2):
                accs = [
                    ps2_pool.tile([P, FREE], f32, name=f"acc{j}", tag="ps2")
                    for j in range(4)
                ]
                for st in range(ST):
                    vt = vld_pool.tile([P, D], bf16, name="vl", tag="vld")
                    nc.sync.dma_start(out=vt[:], in_=vstage[st])
                    for j in range(4):
                        e = g * 4 + j
                        nc.tensor.matmul(
                            accs[j][:],
                            vt[:, ts(e, P)],
                            sct[st][:],
                            start=(st == 0),
                            stop=(st == ST - 1),
                        )
                for j in range(4):
                    a = att_pool.tile([P, FREE], bf16, name="at", tag="att")
                    nc.vector.tensor_copy(a[:], accs[j][:])
                    att[g * 4 + j] = a

            # output projection [t-chunk, 1024]
            for tt in range(FREE // P):
                for nh in range(2):
                    psum = ps_pool.tile([P, FREE], f32, name="pso", tag="ps")
                    for e in range(DT):
                        nc.tensor.matmul(
                            psum[:],
                            att[e][:, ts(tt, P)],
                            wob[e][:, ts(nh, FREE)],
                            start=(e == 0),
                            stop=(e == DT - 1),
                        )
                    ot = ost_pool.tile([P, FREE], f32, name="ot", tag="ost")
                    nc.scalar.copy(ot[:], psum[:])
                    row = c * FREE + tt * P
                    nc.scalar.dma_start(
                        out=out[row : row + P, ts(nh, FREE)], in_=ot[:]
                    )


def _build():
    _install_axon_ntff_shim()
    import concourse.mybir as mybir
    import concourse.tile as tile
    from concourse import bacc

    f32 = mybir.dt.float32
    nc = bacc.Bacc("TRN2", target_bir_lowering=False, debug=False, num_devices=NCORES)
    xT = nc.dram_tensor("xT", [D, T], f32, kind="ExternalInput").ap()
    wq = nc.dram_tensor("wq", [D, D], f32, kind="ExternalInput").ap()
    wk = nc.dram_tensor("wk", [D, D], f32, kind="ExternalInput").ap()
    wv = nc.dram_tensor("wv", [D, D], f32, kind="ExternalInput").ap()
    wo = nc.dram_tensor("wo", [D, D], f32, kind="ExternalInput").ap()
    mask = nc.dram_tensor("mask", [P, 2], f32, kind="ExternalInput").ap()
    out = nc.dram_tensor("out", [H, D], f32, kind="ExternalOutput").ap()

    with tile.TileContext(nc) as tc:
        _trace_kernel(tc, xT, wq, wk, wv, wo, mask, out)
    nc.compile()
    return nc


def kernel(x, w_q, w_k, w_v, w_o):
    global LAST_RESULTS
    from concourse import bass_utils

    if "nc" not in _STATE:
        _STATE["nc"] = _build()
    nc = _STATE["nc"]

    x = np.ascontiguousarray(x, dtype=np.float32)
    in_maps = []
    for core in range(NCORES):
        b, half = core // 2, core % 2
        own = x[b, half * H : (half + 1) * H]
        oth = x[b, (1 - half) * H : (2 - half) * H]
        xT = np.ascontiguousarray(np.concatenate([own, oth], axis=0).T)
        m = np.zeros((P, 2), dtype=np.float32)
        m[:, 1 - half] = 1.0  # zero own slot; pair position == half
        in_maps.append(
            {
                "xT": xT,
                "wq": np.ascontiguousarray(w_q, dtype=np.float32),
                "wk": np.ascontiguousarray(w_k, dtype=np.float32),
                "wv": np.ascontiguousarray(w_v, dtype=np.float32),
                "wo": np.ascontiguousarray(w_o, dtype=np.float32),
                "mask": m,
            }
        )

    LAST_RESULTS = bass_utils.run_bass_kernel_spmd(
        nc, in_maps, core_ids=list(range(NCORES))
    )
    out = np.empty((B, T, D), dtype=np.float32)
    for core in range(NCORES):
        b, half = core // 2, core % 2
        out[b, half * H : (half + 1) * H] = LAST_RESULTS.results[core]["out"]
    return out


# revision 15
# speedup vs baseline: 1.2079x; 1.0485x over previous
"""Trainium2 Bass kernel: unnormalized single-head attention block.

Computes, for x [4, 4096, 1024] and w_q/w_k/w_v/w_o [1024, 1024] (all fp32):
    q = x @ w_q ; k = x @ w_k ; v = x @ w_v
    scores = q @ k.T            (no softmax)
    out = (scores @ v) @ w_o

Sharding: 8 NeuronCores = (4 batches) x (2 sequence halves). Each core
computes the output rows for its 2048-row half of one batch. The host passes
x.T with the core's own half first ("rotated" column order); attention sums
over s are order-independent, so all per-core tensors use that rotated
order consistently.

K projection is computed for the own half only; the peer half arrives via a
masked ReduceScatter over pair groups [[0,1],[2,3],[4,5],[6,7]]: each core
stages its K into both halves of a double buffer scaled by a host-supplied
0/1 mask (own slot zeroed), so the add-reduce-scatter delivers exactly the
peer's K into a uniform buffer on every core -- no rank-dependent addressing
in the SPMD program. V is cheaper to recompute than to exchange on this
fabric (collective transfers are ~100us for 4MB and serialize on the CC
core), so each core projects V over the full rotated sequence.

Device math is bf16 with fp32 PSUM accumulation. Layout chaining (no
on-device transposes anywhere):
    qT[e,t] = wq.T @ x.T        lhsT=wq tile,   rhs=xT
    kT[e,s] = wk.T @ x.T        lhsT=wk tile,   rhs=xT
    v[s,e]  = x @ wv            lhsT=xT tile,   rhs=wv
    sT[s,t] = k @ q.T           lhsT=kT tile,   rhs=qT
    aT[e,t] = v.T @ s           lhsT=v tile,    rhs=sT
    out[t,e]= a @ wo            lhsT=aT tile,   rhs=wo
"""

import contextlib
import ctypes
import os
import sys
import types

import numpy as np

B = 4
T = 4096
D = 1024
H = T // 2          # rows per core
P = 128             # SBUF partitions
NCORES = 8
DT = D // P         # 8 tiles along any 1024 dim
ST = T // P         # 32 tiles along the full sequence
STH = H // P        # 16 own-half s-tiles
FREE = 512          # matmul moving free dim / PSUM bank width (fp32)
SBLK = T // FREE    # 8 full-sequence blocks of 512
CH = H // FREE      # 4 t-chunks per core
GROUPS = [[0, 1], [2, 3], [4, 5], [6, 7]]

_STATE = {}
LAST_RESULTS = None


def _install_axon_ntff_shim():
    """bass_utils(trace=True) under axon imports antenv.axon_hooks, which the
    agent image lacks. Provide the documented ctypes equivalent so tracing
    works; degrades to hook=None when the .so has no profile symbols."""
    try:
        import antenv.axon_hooks  # noqa: F401
        return
    except ImportError:
        pass

    so_path = "/opt/axon/libaxon_pjrt.so"

    def _make_hook():
        try:
            lib = ctypes.CDLL(so_path)
        except OSError:
            return None
        if not hasattr(lib, "axon_start_nrt_profile"):
            return None
        lib.axon_start_nrt_profile.argtypes = [
            ctypes.POINTER(ctypes.c_int64),
            ctypes.c_size_t,
        ]
        lib.axon_start_nrt_profile.restype = ctypes.c_int64
        lib.axon_stop_nrt_profile.argtypes = [ctypes.c_char_p]
        lib.axon_stop_nrt_profile.restype = ctypes.c_int64

        @contextlib.contextmanager
        def _hook(output_dir, device_ids):
            import jax

            jax.devices()
            if device_ids:
                ids = (ctypes.c_int64 * len(device_ids))(*device_ids)
                rc = lib.axon_start_nrt_profile(ids, len(device_ids))
            else:
                rc = lib.axon_start_nrt_profile(None, 0)
            if rc != 0:
                raise RuntimeError(f"axon_start_nrt_profile rc={rc}")
            try:
                yield
            finally:
                n = lib.axon_stop_nrt_profile(str(output_dir).encode())
                print(f"profile: {n} file(s) written to {output_dir}", file=sys.stderr)

        return _hook

    mod = types.ModuleType("antenv.axon_hooks")
    mod.get_axon_ntff_profile_hook = _make_hook
    mod.set_axon_ntff_profile_hook = lambda h: None
    sys.modules["antenv.axon_hooks"] = mod


def _trace_kernel(tc, xT, wq, wk, wv, wo, mask, out):
    import concourse.mybir as mybir
    from concourse.bass import ts

    nc = tc.nc
    f32 = mybir.dt.float32
    bf16 = mybir.dt.bfloat16

    with contextlib.ExitStack() as top:
        # Long-lived pools
        ktr_pool = top.enter_context(tc.tile_pool(name="ktr", bufs=DT))
        ktb_pool = top.enter_context(tc.tile_pool(name="ktb", bufs=DT))
        qt_pool = top.enter_context(tc.tile_pool(name="qt", bufs=DT))
        const_pool = top.enter_context(tc.tile_pool(name="cst", bufs=1))
        ps_pool = top.enter_context(tc.tile_pool(name="ps", bufs=8, space="PSUM"))
        dram_pool = top.enter_context(tc.tile_pool(name="cdram", bufs=4, space="DRAM"))

        # Own-half kT in row layout (filled straight from PSUM evictions);
        # peer-half kT in st-blocked layout (one DMA per s-tile from kpeer).
        ktr = [
            ktr_pool.tile([P, H], bf16, name=f"ktr{i}", tag="ktr") for i in range(DT)
        ]
        ktb = [
            ktb_pool.tile([P, H], bf16, name=f"ktb{i}", tag="ktb") for i in range(DT)
        ]
        qt = [qt_pool.tile([P, H], bf16, name=f"qt{i}", tag="qt") for i in range(DT)]

        mb = const_pool.tile([P, 2], f32, name="mb", tag="mb")
        nc.sync.dma_start(out=mb[:], in_=mask)

        # K-collective staging (2-core groups need Local addr space) and the
        # full-sequence V staging in local DRAM.
        kstage = dram_pool.tile([2, DT, P, H], bf16, name="kstage", tag="kst")
        kpeer = dram_pool.tile([DT, P, H], bf16, name="kpeer", tag="kp")
        vstage = dram_pool.tile([ST, P, D], bf16, name="vstage", tag="vso")

        # ---------------- setup ----------------
        with contextlib.ExitStack() as setup:
            wf_pool = setup.enter_context(tc.tile_pool(name="wf", bufs=2))
            wset_pool = setup.enter_context(tc.tile_pool(name="wset", bufs=2 * DT))
            xf_pool = setup.enter_context(tc.tile_pool(name="xf", bufs=4))
            xb_pool = setup.enter_context(tc.tile_pool(name="xb", bufs=2 * DT))
            esb_pool = setup.enter_context(tc.tile_pool(name="esb", bufs=6))

            def cast_weight(w_ap):
                tiles = []
                for i in range(DT):
                    wf = wf_pool.tile([P, D], f32, name="wf", tag="wf")
                    nc.sync.dma_start(out=wf[:], in_=w_ap[ts(i, P), :])
                    wb = wset_pool.tile([P, D], bf16, name="wb", tag="wset")
                    nc.scalar.copy(wb[:], wf[:])
                    tiles.append(wb)
                return tiles

            def load_x_block(blk):
                xb = []
                for d in range(DT):
                    xf = xf_pool.tile([P, FREE], f32, name="xf", tag="xf")
                    nc.sync.dma_start(out=xf[:], in_=xT[ts(d, P), ts(blk, FREE)])
                    xbt = xb_pool.tile([P, FREE], bf16, name="xbt", tag="xb")
                    nc.scalar.copy(xbt[:], xf[:])
                    xb.append(xbt)
                return xb

            # --- K pass (own half): fill ktr + masked staging for the RS ---
            wkb = cast_weight(wk)
            for blk in range(CH):
                xb = load_x_block(blk)
                for e in range(DT):
                    psum = ps_pool.tile([P, FREE], f32, name="psk", tag="ps")
                    for d in range(DT):
                        nc.tensor.matmul(
                            psum[:],
                            wkb[d][:, ts(e, P)],
                            xb[d][:],
                            start=(d == 0),
                            stop=(d == DT - 1),
                        )
                    nc.vector.tensor_copy(ktr[e][:, ts(blk, FREE)], psum[:])
                    for part in range(2):
                        km = esb_pool.tile([P, FREE], bf16, name="km", tag="esbk")
                        nc.vector.tensor_scalar_mul(
                            km[:], ktr[e][:, ts(blk, FREE)], mb[:, part : part + 1]
                        )
                        # Stores ride the scalar HWDGE queue so they do not
                        # back up the sync queue feeding the x loads.
                        nc.scalar.dma_start(
                            out=kstage[part, e, :, ts(blk, FREE)], in_=km[:]
                        )
            nc.gpsimd.collective_compute(
                "ReduceScatter",
                mybir.AluOpType.add,
                replica_groups=GROUPS,
                ins=[kstage.opt()],
                outs=[kpeer.opt()],
            )

            # --- combined V (full sequence) + Q (own half) pass ---
            wvb = cast_weight(wv)
            wqb = cast_weight(wq)
            for blk in range(SBLK):
                xb = load_x_block(blk)
                for ss in range(FREE // P):
                    s_tile = blk * (FREE // P) + ss
                    vt = esb_pool.tile([P, D], bf16, name="vt", tag="esbv")
                    for nh in range(2):
                        psum = ps_pool.tile([P, FREE], f32, name="psv", tag="ps")
                        for d in range(DT):
                            nc.tensor.matmul(
                                psum[:],
                                xb[d][:, ts(ss, P)],
                                wvb[d][:, ts(nh, FREE)],
                                start=(d == 0),
                                stop=(d == DT - 1),
                            )
                        nc.vector.tensor_copy(vt[:, ts(nh, FREE)], psum[:])
                    nc.scalar.dma_start(out=vstage[s_tile], in_=vt[:])
                if blk < CH:  # q projection for the own half
                    for e in range(DT):
                        psum = ps_pool.tile([P, FREE], f32, name="psq", tag="ps")
                        for d in range(DT):
                            nc.tensor.matmul(
                                psum[:],
                                wqb[d][:, ts(e, P)],
                                xb[d][:],
                                start=(d == 0),
                                stop=(d == DT - 1),
                            )
                        nc.vector.tensor_copy(qt[e][:, ts(blk, FREE)], psum[:])

        # Peer-half kT into SBUF (waits on the K ReduceScatter via tile
        # deps). Issued from the otherwise-idle SWDGE queue so the wait on
        # the collective cannot stall the sync/scalar DMA queues.
        for e in range(DT):
            nc.gpsimd.dma_start(out=ktb[e][:], in_=kpeer[e])

        # w_o cast (after setup pools release)
        wo_pool = top.enter_context(tc.tile_pool(name="wob", bufs=DT))
        wof_pool = top.enter_context(tc.tile_pool(name="wof", bufs=2))
        wob = []
        for i in range(DT):
            wf = wof_pool.tile([P, D], f32, name="wof", tag="wof")
            nc.sync.dma_start(out=wf[:], in_=wo[ts(i, P), :])
            wb = wo_pool.tile([P, D], bf16, name="wob", tag="wob")
            nc.scalar.copy(wb[:], wf[:])
            wob.append(wb)

        # ---------------- main loop over t-chunks ----------------
        sct_pool = top.enter_context(tc.tile_pool(name="sct", bufs=ST))
        att_pool = top.enter_context(tc.tile_pool(name="att", bufs=2 * DT))
        vld_pool = top.enter_context(tc.tile_pool(name="vld", bufs=6))
        ost_pool = top.enter_context(tc.tile_pool(name="ost", bufs=4))

        for c in range(CH):
            # scores^T [s, t-chunk]: own half from ktr, peer half from ktb
            sct = []
            for st in range(ST):
                psum = ps_pool.tile([P, FREE], f32, name="pss", tag="ps")
                for e in range(DT):
                    lhsT = (
                        ktr[e][:, ts(st, P)]
                        if st < STH
                        else ktb[e][:, ts(st - STH, P)]
                    )
                    nc.tensor.matmul(
                        psum[:],
                        lhsT,
                        qt[e][:, ts(c, FREE)],
                        start=(e == 0),
                        stop=(e == DT - 1),
                    )
                sc = sct_pool.tile([P, FREE], bf16, name="sc", tag="sct")
                nc.vector.tensor_copy(sc[:], psum[:])
                sct.append(sc)

            # attn^T [e, t-chunk]: all 8 PSUM banks accumulate over s, so v
            # streams through SBUF exactly once per chunk.
            att = [None] * DT
            accs = [
                ps_pool.tile([P, FREE], f32, name=f"acc{j}", tag="ps")
                for j in range(DT)
            ]
            for st in range(ST):
                vt = vld_pool.tile([P, D], bf16, name="vl", tag="vld")
                nc.sync.dma_start(out=vt[:], in_=vstage[st])
                for e in range(DT):
                    nc.tensor.matmul(
                        accs[e][:],
                        vt[:, ts(e, P)],
                        sct[st][:],
                        start=(st == 0),
                        stop=(st == ST - 1),
                    )
            for e in range(DT):
                a = att_pool.tile([P, FREE], bf16, name="at", tag="att")
                nc.vector.tensor_copy(a[:], accs[e][:])
                att[e] = a

            # output projection [t-chunk, 1024]
            for tt in range(FREE // P):
                for nh in range(2):
                    psum = ps_pool.tile([P, FREE], f32, name="pso", tag="ps")
                    for e in range(DT):
                        nc.tensor.matmul(
                            psum[:],
                            att[e][:, ts(tt, P)],
                            wob[e][:, ts(nh, FREE)],
                            start=(e == 0),
                            stop=(e == DT - 1),
                        )
                    ot = ost_pool.tile([P, FREE], f32, name="ot", tag="ost")
                    nc.scalar.copy(ot[:], psum[:])
                    row = c * FREE + tt * P
                    nc.scalar.dma_start(
                        out=out[row : row + P, ts(nh, FREE)], in_=ot[:]
                    )


def _build():
    _install_axon_ntff_shim()
    import concourse.mybir as mybir
    import concourse.tile as tile
    from concourse import bacc

    f32 = mybir.dt.float32
    nc = bacc.Bacc("TRN2", target_bir_lowering=False, debug=False, num_devices=NCORES)
    xT = nc.dram_tensor("xT", [D, T], f32, kind="ExternalInput").ap()
    wq = nc.dram_tensor("wq", [D, D], f32, kind="ExternalInput").ap()
    wk = nc.dram_tensor("wk", [D, D], f32, kind="ExternalInput").ap()
    wv = nc.dram_tensor("wv", [D, D], f32, kind="ExternalInput").ap()
    wo = nc.dram_tensor("wo", [D, D], f32, kind="ExternalInput").ap()
    mask = nc.dram_tensor("mask", [P, 2], f32, kind="ExternalInput").ap()
    out = nc.dram_tensor("out", [H, D], f32, kind="ExternalOutput").ap()

    with tile.TileContext(nc) as tc:
        _trace_kernel(tc, xT, wq, wk, wv, wo, mask, out)
    nc.compile()
    return nc


def kernel(x, w_q, w_k, w_v, w_o):
    global LAST_RESULTS
    from concourse import bass_utils

    if "nc" not in _STATE:
        _STATE["nc"] = _build()
    nc = _STATE["nc"]

    x = np.ascontiguousarray(x, dtype=np.float32)
    in_maps = []
    for core in range(NCORES):
        b, half = core // 2, core % 2
        own = x[b, half * H : (half + 1) * H]
        oth = x[b, (1 - half) * H : (2 - half) * H]
        xT = np.ascontiguousarray(np.concatenate([own, oth], axis=0).T)
        m = np.zeros((P, 2), dtype=np.float32)
        m[:, 1 - half] = 1.0  # zero own slot; pair position == half
        in_maps.append(
            {
                "xT": xT,
                "wq": np.ascontiguousarray(w_q, dtype=np.float32),
                "wk": np.ascontiguousarray(w_k, dtype=np.float32),
                "wv": np.ascontiguousarray(w_v, dtype=np.float32),
                "wo": np.ascontiguousarray(w_o, dtype=np.float32),
                "mask": m,
            }
        )

    LAST_RESULTS = bass_utils.run_bass_kernel_spmd(
        nc, in_maps, core_ids=list(range(NCORES))
    )
    out = np.empty((B, T, D), dtype=np.float32)
    for core in range(NCORES):
        b, half = core // 2, core % 2
        out[b, half * H : (half + 1) * H] = LAST_RESULTS.results[core]["out"]
    return out
